# revision 14
# baseline (speedup 1.0000x reference)
"""Trainium2 Bass kernel for nn_MultiHeadAttention_73675868995934.

8-core sharding: core c handles batch b=c//2, sequence-half sh=c%2 (1024 query
rows). No collectives: each core computes full K/V projections for its batch
(duplicated within the core pair), all 8 heads for its query half, the fc
projection, residual and LayerNorm for its rows.

Reference quirks faithfully reproduced:
  - V is projected with W_K (not W_V).
  - [B,S,H*d] -> [B,H,S,d] reshape without transpose: head h's Q matrix
    [2048, 64] is Qproj[b, h*256:(h+1)*256, :].reshape(2048, 64), i.e. query
    s of head h lives at Qproj row h*256 + s//8, columns 64*(s%8) +: 64.
    In-kernel we use the permuted index q~=(j, t), s_local = 8t + j, and the
    analogous k~=(j', u), k = 8u + j'.

Softmax has no max-subtraction (scores are O(6) so exp cannot overflow) and
the mask is applied multiplicatively after exp (exact: masked ref scores are
-1e9 whose exp underflows to 0).

Scores are computed twice: once as S [q-part, k-free] for the attn output
(natural, contiguous stores) and once as S^T [k-part, q-free] to feed the
attn@V contraction (PE contracts over the partition dim). This is cheaper
than any transpose path for the 16.8M-element P matrix per core.
"""

import sys

sys.path.insert(0, "/opt/trn_rl_repo")

import numpy as np

import concourse.bass as bass
import concourse.mybir as mybir
import concourse.tile as tile
from concourse import bacc
from concourse.bass import ds
from concourse.masks import make_identity

FP32 = mybir.dt.float32
BF16 = mybir.dt.bfloat16
U8 = mybir.dt.uint8
AF = mybir.ActivationFunctionType
OP = mybir.AluOpType

H = 8          # heads
DH = 64        # head dim
DM = 512       # model dim
SQ = 1024      # query rows per core
SK = 2048      # key rows
LN_EPS = 1e-5
SCALE = 0.125  # 1/sqrt(64)


def build(nc: bass.Bass):
    xq_proj = nc.dram_tensor("xq_proj", [SQ, DM], FP32, kind="ExternalInput")
    xq_resid = nc.dram_tensor("xq_resid", [SQ, DM], FP32, kind="ExternalInput")
    xk = nc.dram_tensor("xk", [SK, DM], FP32, kind="ExternalInput")
    xv = nc.dram_tensor("xv", [SK, DM], FP32, kind="ExternalInput")
    maskh = nc.dram_tensor("maskh", [SQ, SK], U8, kind="ExternalInput")
    wq = nc.dram_tensor("wq", [DM, DM], FP32, kind="ExternalInput")
    wk = nc.dram_tensor("wk", [DM, DM], FP32, kind="ExternalInput")
    wfc = nc.dram_tensor("wfc", [DM, DM], FP32, kind="ExternalInput")
    gamma = nc.dram_tensor("gamma", [DM, 1], FP32, kind="ExternalInput")
    beta = nc.dram_tensor("beta", [DM, 1], FP32, kind="ExternalInput")
    attn_o = nc.dram_tensor("attn_o", [H, SQ, SK], FP32, kind="ExternalOutput")
    normed_o = nc.dram_tensor("normed_o", [SQ, DM], FP32, kind="ExternalOutput")

    with tile.TileContext(nc) as tc:
        _body(nc, tc, xq_proj, xq_resid, xk, xv, maskh, wq, wk, wfc,
              gamma, beta, attn_o, normed_o)
    return nc


def _body(nc, tc, xq_proj, xq_resid, xk, xv, maskh, wq, wk, wfc,
          gamma, beta, attn_o, normed_o):
    import contextlib
    ctx = contextlib.ExitStack()
    with ctx:
        cst = ctx.enter_context(tc.tile_pool(name="cst", bufs=1))
        work = ctx.enter_context(tc.tile_pool(name="work", bufs=2))
        head = ctx.enter_context(tc.tile_pool(name="head", bufs=2))
        ps_big = ctx.enter_context(tc.tile_pool(name="psb", bufs=2, space="PSUM"))
        ps_small = ctx.enter_context(tc.tile_pool(name="pss", bufs=4, space="PSUM"))

        # ---------------- constants ----------------
        id_bf = cst.tile([128, 128], BF16)
        id_f32 = cst.tile([128, 128], FP32)
        make_identity(nc, id_bf)
        make_identity(nc, id_f32)
        ones_col = cst.tile([128, 1], FP32)   # lhsT for partition-sum
        nc.vector.memset(ones_col, 1.0)
        ones_row = cst.tile([1, 128], FP32)   # lhsT for partition-broadcast
        nc.vector.memset(ones_row, 1.0)

        g_sb = cst.tile([128, 4, 1], FP32)
        b_sb = cst.tile([128, 4, 1], FP32)
        for cc in range(4):
            nc.sync.dma_start(g_sb[:, cc, :], gamma[ds(cc * 128, 128), :])
            nc.sync.dma_start(b_sb[:, cc, :], beta[ds(cc * 128, 128), :])

        # ---------------- weight transposes (bf16) ----------------
        def load_wT(w_dram, name):
            # wT[m, mc, o]
            wT = cst.tile([128, 4, DM], BF16, tag=name, name=name)
            for oc in range(4):
                w_natf = work.tile([128, DM], FP32, tag="natf", name="w_natf")
                nc.sync.dma_start(w_natf, w_dram[ds(oc * 128, 128), :])
                w_nat = work.tile([128, DM], BF16, tag="nat", name="w_nat")
                nc.vector.tensor_copy(w_nat, w_natf)
                for mc in range(4):
                    psT = ps_small.tile([128, 128], BF16, tag="small", name="psT")
                    nc.tensor.transpose(psT, w_nat[:, ds(mc * 128, 128)], id_bf)
                    nc.any.tensor_copy(wT[:, mc, ds(oc * 128, 128)], psT)
            return wT

        wqT = load_wT(wq, "wqT")
        wkT = load_wT(wk, "wkT")
        wfcT = load_wT(wfc, "wfcT")

        # ---------------- mask prep ----------------
        # mask01[t, j, k] = 1 - mask  (bf16); q~ tile j holds rows s=8t+j
        m01 = cst.tile([128, H, SK], BF16)
        maskh_v = maskh.rearrange("(t j) k -> t j k", j=8)
        for j in range(H):
            mu8 = work.tile([128, SK], U8, tag="pt_raw", name="mu8")
            nc.sync.dma_start(mu8, maskh_v[:, j, :])
            mbf = work.tile([128, SK], BF16, tag="p_raw", name="mbf")
            nc.vector.tensor_copy(mbf, mu8)
            nc.vector.tensor_scalar(m01[:, j, :], mbf, -1.0, 1.0, OP.mult, OP.add)

        # maskT01 [u, kt, q~]  (kt = j'*2 + ut)
        maskT = cst.tile([128, 16, SQ], BF16)
        for jp in range(8):
            for ut in range(2):
                kt = jp * 2 + ut
                for j in range(8):
                    src = m01[:, j, :].rearrange("t (u jp) -> t jp u", jp=8)
                    psT = ps_small.tile([128, 128], BF16, tag="small", name="psT")
                    nc.tensor.transpose(
                        psT, src[:, jp, ds(ut * 128, 128)], id_bf)
                    nc.any.tensor_copy(maskT[:, kt, ds(j * 128, 128)], psT)

        # ---------------- per-core persistent ----------------
        ctxT = cst.tile([128, 4, SQ], BF16)     # context^T [c, cc, q~]
        xT = cst.tile([128, 4, SQ], FP32)       # fc out + residual, [c, cc, q~]

        xq_proj_v = xq_proj.rearrange("(h t) c -> h t c", h=8)
        xk_v = xk.rearrange("(h r t) c -> h r t c", h=8, r=2)
        xv_v = xv.rearrange("(h r t) c -> h r t c", h=8, r=2)

        # ================= head loop =================
        for h in range(H):
            # ---- transposed input blocks ----
            xqT = head.tile([128, 4, 128], BF16, tag="xqT", name="xqT", bufs=1)
            xq_natf = work.tile([128, DM], FP32, tag="natf", name="xq_natf")
            nc.sync.dma_start(xq_natf, xq_proj_v[h])
            xq_nat = work.tile([128, DM], BF16, tag="nat", name="xq_nat")
            nc.vector.tensor_copy(xq_nat, xq_natf)
            for mc in range(4):
                psT = ps_small.tile([128, 128], BF16, tag="small", name="psT")
                nc.tensor.transpose(psT, xq_nat[:, ds(mc * 128, 128)], id_bf)
                nc.any.tensor_copy(xqT[:, mc, :], psT)

            xkT = head.tile([128, 4, 256], BF16, tag="xkT", name="xkT", bufs=1)
            xvT = head.tile([128, 4, 256], BF16, tag="xvT", name="xvT", bufs=1)
            for src_v, dstT in ((xk_v, xkT), (xv_v, xvT)):
                for rt in range(2):
                    natf = work.tile([128, DM], FP32, tag="natf", name="kv_natf")
                    nc.sync.dma_start(natf, src_v[h, rt])
                    nat = work.tile([128, DM], BF16, tag="nat", name="kv_nat")
                    nc.vector.tensor_copy(nat, natf)
                    for mc in range(4):
                        psT = ps_small.tile([128, 128], BF16, tag="small",
                                            name="psT")
                        nc.tensor.transpose(psT, nat[:, ds(mc * 128, 128)], id_bf)
                        nc.any.tensor_copy(dstT[:, mc, ds(rt * 128, 128)], psT)

            # ---- projections (64-partition tiles, base 0, j in free dim) ----
            QpT = head.tile([64, 8, 128], BF16, tag="QpT", name="QpT")
            for j in range(8):
                ps = ps_small.tile([64, 128], FP32, tag="small", name="ps_q")
                for mc in range(4):
                    nc.tensor.matmul(ps, wqT[:, mc, ds(j * 64, 64)],
                                     xqT[:, mc, :],
                                     start=(mc == 0), stop=(mc == 3))
                nc.any.tensor_copy(QpT[:, j, :], ps)

            KpT = head.tile([64, 8, 256], BF16, tag="KpT", name="KpT")
            for jp in range(8):
                ps = ps_small.tile([64, 256], FP32, tag="small", name="ps_k")
                for mc in range(4):
                    nc.tensor.matmul(ps, wkT[:, mc, ds(jp * 64, 64)],
                                     xkT[:, mc, :],
                                     start=(mc == 0), stop=(mc == 3))
                nc.any.tensor_copy(KpT[:, jp, :], ps)

            Vp = head.tile([128, 2, DM], BF16, tag="Vp", name="Vp")
            for ut in range(2):
                ps = ps_big.tile([128, 1024], FP32, tag="big", name="ps_v")
                for mc in range(4):
                    nc.tensor.matmul(ps[:, 0:DM], xvT[:, mc, ds(ut * 128, 128)],
                                     wkT[:, mc, :],
                                     start=(mc == 0), stop=(mc == 3))
                nc.any.tensor_copy(Vp[:, ut, :], ps[:, 0:DM])

            vones = head.tile([128, 2, 8, 65], BF16, tag="vones", name="vones", bufs=1)
            nc.vector.memset(vones[:, :, :, 64:65], 1.0)
            for ut in range(2):
                for jp in range(8):
                    nc.any.tensor_copy(vones[:, ut, jp, 0:64],
                                       Vp[:, ut, ds(jp * 64, 64)])

            # ---- S^T phase: PT_m[u, kt, q~] = exp(S^T)*maskT ----
            PT_m = head.tile([128, 16, SQ], BF16, tag="PT_m", name="PT_m",
                             bufs=1)
            for jp in range(8):
                for ut in range(2):
                    kt = jp * 2 + ut
                    ps = ps_big.tile([128, 1024], FP32, tag="big", name="ps_st")
                    for j in range(8):
                        nc.tensor.matmul(
                            ps[:, ds(j * 128, 128)],
                            KpT[:, jp, ds(ut * 128, 128)],
                            QpT[:, j, :],
                            start=True, stop=True)
                    pt_raw = work.tile([128, SQ], BF16, tag="pt_raw",
                                       name="pt_raw")
                    nc.scalar.activation(pt_raw, ps, AF.Exp, scale=SCALE)
                    nc.vector.tensor_tensor(PT_m[:, kt, :], pt_raw,
                                            maskT[:, kt, :], OP.mult)

            # ---- AV + denominators + context^T ----
            recs = head.tile([128, 8, 1], FP32, tag="recs", name="recs")
            for j in range(8):
                ps_av = ps_small.tile([128, 65], FP32, tag="small", name="ps_av")
                for kt in range(16):
                    jp, ut = kt // 2, kt % 2
                    nc.tensor.matmul(ps_av, PT_m[:, kt, ds(j * 128, 128)],
                                     vones[:, ut, jp, :],
                                     start=(kt == 0), stop=(kt == 15))
                dcol = work.tile([128, 1], FP32, tag="dcol", name="dcol")
                nc.vector.tensor_copy(dcol, ps_av[:, 64:65])
                nc.vector.reciprocal(recs[:, j, :], dcol)
                ctx_t = work.tile([128, 64], BF16, tag="ctx_t", name="ctx_t")
                nc.vector.tensor_scalar(ctx_t, ps_av[:, 0:64], recs[:, j, :],
                                        None, OP.mult)
                hp = 64 * (h % 2)
                psT = ps_small.tile([128, 128], BF16, tag="small", name="psTc")
                nc.tensor.transpose(psT[hp:hp + 64, :], ctx_t, id_bf,
                                    tile_position=(0, hp))
                nc.any.tensor_copy(
                    ctxT[hp:hp + 64, h // 2, ds(j * 128, 128)],
                    psT[hp:hp + 64, :])

            # ---- S phase (natural-k): attn rows out ----
            for j in range(8):
                p_raw = work.tile([128, SK], BF16, tag="p_raw", name="p_raw")
                p_raw_v = p_raw.rearrange("t (u jp) -> t jp u", jp=8)
                for half in range(2):
                    ps = ps_big.tile([128, 1024], FP32, tag="big", name="ps_s")
                    for jj in range(4):
                        jp = half * 4 + jj
                        nc.tensor.matmul(
                            ps[:, ds(jj * 256, 256)],
                            QpT[:, j, :],
                            KpT[:, jp, :],
                            start=True, stop=True)
                    ps_v = ps.rearrange("t (jj u) -> t jj u", jj=4)
                    nc.scalar.activation(p_raw_v[:, ds(half * 4, 4), :], ps_v,
                                         AF.Exp, scale=SCALE)
                # fused: attn = (p_raw * recip) * mask01, bf16
                p_mn = work.tile([128, SK], BF16, tag="p_mn", name="p_mn")
                nc.vector.scalar_tensor_tensor(p_mn, p_raw, recs[:, j, :],
                                               m01[:, j, :], OP.mult, OP.mult)
                # cast-store bf16 -> f32 via SWDGE
                nc.gpsimd.dma_start(
                    attn_o[h].rearrange("(t j) k -> t j k", j=8)[:, j, :],
                    p_mn)

        # ================= epilogue =================
        # residual rows, transposed: xqrT [c, cc, q~] (tag-shares PT_m slot)
        xqrT = head.tile([128, 4, SQ], BF16, tag="PT_m", name="xqrT", bufs=1)
        xqr_v = xq_resid.rearrange("(t j) c -> t j c", j=8)
        for j in range(8):
            xr_natf = work.tile([128, DM], FP32, tag="natf", name="xr_natf")
            nc.sync.dma_start(xr_natf, xqr_v[:, j, :])
            xr_nat = work.tile([128, DM], BF16, tag="nat", name="xr_nat")
            nc.vector.tensor_copy(xr_nat, xr_natf)
            for cc in range(4):
                psT = ps_small.tile([128, 128], BF16, tag="small", name="psT")
                nc.tensor.transpose(psT, xr_nat[:, ds(cc * 128, 128)], id_bf)
                nc.any.tensor_copy(xqrT[:, cc, ds(j * 128, 128)], psT)

        # fc + residual -> xT
        for oc in range(4):
            ps = ps_big.tile([128, 1024], FP32, tag="big", name="ps_fc")
            for cc in range(4):
                for qh in range(2):
                    nc.tensor.matmul(ps[:, ds(qh * 512, 512)],
                                     wfcT[:, cc, ds(oc * 128, 128)],
                                     ctxT[:, cc, ds(qh * 512, 512)],
                                     start=(cc == 0), stop=(cc == 3))
            nc.vector.tensor_tensor(xT[:, oc, :], ps, xqrT[:, oc, :], OP.add)

        # LN stats: sum(x), sum(x^2) over c via ones-matmul (interleaved groups)
        mv = cst.tile([1, SQ], FP32)
        var = cst.tile([1, SQ], FP32)
        rstd = cst.tile([1, SQ], FP32)
        ps_s1a = ps_small.tile([1, 512], FP32, tag="small", name="ps_s1a")
        ps_s1b = ps_small.tile([1, 512], FP32, tag="small", name="ps_s1b")
        ps_s2a = ps_small.tile([1, 512], FP32, tag="small", name="ps_s2a")
        ps_s2b = ps_small.tile([1, 512], FP32, tag="small", name="ps_s2b")
        for cc in range(4):
            x2c = work.tile([128, SQ], FP32, tag="x2c", name="x2c")
            nc.vector.tensor_tensor(x2c, xT[:, cc, :], xT[:, cc, :], OP.mult)
            st = (cc == 0)
            sp = (cc == 3)
            nc.tensor.matmul(ps_s1a, ones_col, xT[:, cc, 0:512], start=st, stop=sp)
            nc.tensor.matmul(ps_s1b, ones_col, xT[:, cc, 512:1024], start=st,
                             stop=sp)
            nc.tensor.matmul(ps_s2a, ones_col, x2c[:, 0:512], start=st, stop=sp)
            nc.tensor.matmul(ps_s2b, ones_col, x2c[:, 512:1024], start=st,
                             stop=sp)
        nc.vector.tensor_scalar(mv[:, 0:512], ps_s1a, 1.0 / DM, None, OP.mult)
        nc.vector.tensor_scalar(mv[:, 512:1024], ps_s1b, 1.0 / DM, None, OP.mult)
        nc.vector.tensor_scalar(var[:, 0:512], ps_s2a, 1.0 / DM, None, OP.mult)
        nc.vector.tensor_scalar(var[:, 512:1024], ps_s2b, 1.0 / DM, None,
                                OP.mult)
        m2 = work.tile([1, SQ], FP32, tag="m2", name="m2", bufs=1)
        nc.vector.tensor_tensor(m2, mv, mv, OP.mult)
        nc.vector.tensor_tensor(var, var, m2, OP.subtract)
        nc.vector.tensor_scalar(var, var, LN_EPS, None, OP.add)
        sqv = work.tile([1, SQ], FP32, tag="m2", name="sqv", bufs=1)
        nc.scalar.activation(sqv, var, AF.Sqrt)
        nc.vector.reciprocal(rstd, sqv)

        # broadcast mean/rstd across partitions via PE outer product
        ps_bm = ps_big.tile([128, 1024], FP32, tag="big", name="ps_bm")
        ps_br = ps_big.tile([128, 1024], FP32, tag="big", name="ps_br")
        for qh in range(2):
            nc.tensor.matmul(ps_bm[:, ds(qh * 512, 512)], ones_row,
                             mv[:, ds(qh * 512, 512)], start=True, stop=True)
            nc.tensor.matmul(ps_br[:, ds(qh * 512, 512)], ones_row,
                             rstd[:, ds(qh * 512, 512)], start=True, stop=True)

        # apply LN, transpose back, store
        normed_v = normed_o.rearrange("(t j) c -> t j c", j=8)
        for cc in range(4):
            t1 = work.tile([128, SQ], FP32, tag="x2c", name="t1")
            nc.vector.tensor_tensor(t1, xT[:, cc, :], ps_bm, OP.subtract)
            nc.vector.tensor_tensor(t1, t1, ps_br, OP.mult)
            nc.vector.tensor_scalar(t1, t1, g_sb[:, cc, :], b_sb[:, cc, :],
                                    OP.mult, OP.add)
            for j in range(8):
                psT = ps_small.tile([128, 128], FP32, tag="small", name="psTn")
                nc.tensor.transpose(psT, t1[:, ds(j * 128, 128)], id_f32)
                nn = work.tile([128, 128], FP32, tag="nn", name="nn")
                nc.vector.tensor_copy(nn, psT)
                nc.sync.dma_start(normed_v[:, j, ds(cc * 128, 128)], nn)


# ---------------------------------------------------------------------------
# host side
# ---------------------------------------------------------------------------
_CACHED = {}


def _get_nc():
    if "nc" not in _CACHED:
        nc = bacc.Bacc("TRN2", target_bir_lowering=False)
        build(nc)
        nc.compile()
        _CACHED["nc"] = nc
    return _CACHED["nc"]


def kernel(input_Q, input_K, input_V, attention_mask, W_Q, W_K, W_V, W_fc,
           ln_gamma, ln_beta):
    from concourse import bass_utils

    input_Q = np.asarray(input_Q, dtype=np.float32)
    input_K = np.asarray(input_K, dtype=np.float32)
    input_V = np.asarray(input_V, dtype=np.float32)
    mask_u8 = np.asarray(attention_mask).astype(np.uint8)
    W_Q = np.ascontiguousarray(np.asarray(W_Q, dtype=np.float32))
    W_K = np.ascontiguousarray(np.asarray(W_K, dtype=np.float32))
    W_fc = np.ascontiguousarray(np.asarray(W_fc, dtype=np.float32))
    gamma = np.ascontiguousarray(
        np.asarray(ln_gamma, dtype=np.float32).reshape(DM, 1))
    beta = np.ascontiguousarray(
        np.asarray(ln_beta, dtype=np.float32).reshape(DM, 1))

    nc = _get_nc()
    in_maps = []
    for c in range(8):
        b, sh = c // 2, c % 2
        xq = input_Q[b]
        in_maps.append({
            "xq_proj": np.ascontiguousarray(
                xq.reshape(8, 2, 128, DM)[:, sh].reshape(SQ, DM)),
            "xq_resid": np.ascontiguousarray(xq[sh * SQ:(sh + 1) * SQ]),
            "xk": np.ascontiguousarray(input_K[b]),
            "xv": np.ascontiguousarray(input_V[b]),
            "maskh": np.ascontiguousarray(mask_u8[b, sh * SQ:(sh + 1) * SQ]),
            "wq": W_Q, "wk": W_K, "wfc": W_fc,
            "gamma": gamma, "beta": beta,
        })

    trace = bool(globals().get("_TRACE", False))
    kw = {}
    if trace:
        kw = dict(trace=True, tmpdir="/root/problem/profile")
    res = bass_utils.run_bass_kernel_spmd(nc, in_maps, core_ids=list(range(8)),
                                          **kw)
    results = res.results
    _CACHED["exec_time_ns"] = res.exec_time_ns
    _CACHED["profile_json"] = getattr(res, "profile_json", None)

    B, S = 4, 2048
    normed = np.empty((B, S, DM), dtype=np.float32)
    attn = np.empty((B, H, S, S), dtype=np.float32)
    for c in range(8):
        b, sh = c // 2, c % 2
        normed[b, sh * SQ:(sh + 1) * SQ] = results[c]["normed_o"]
        attn[b, :, sh * SQ:(sh + 1) * SQ, :] = results[c]["attn_o"]
    return normed, attn


# revision 17
# speedup vs baseline: 1.0450x; 1.0450x over previous
"""Trainium2 Bass kernel for nn_MultiHeadAttention_73675868995934.

8-core sharding: core c handles batch b=c//2, sequence-half sh=c%2 (1024 query
rows). No collectives: each core computes full K/V projections for its batch
(duplicated within the core pair), all 8 heads for its query half, the fc
projection, residual and LayerNorm for its rows.

Reference quirks faithfully reproduced:
  - V is projected with W_K (not W_V).
  - [B,S,H*d] -> [B,H,S,d] reshape without transpose: head h's Q matrix
    [2048, 64] is Qproj[b, h*256:(h+1)*256, :].reshape(2048, 64), i.e. query
    s of head h lives at Qproj row h*256 + s//8, columns 64*(s%8) +: 64.
    In-kernel we use the permuted index q~=(j, t), s_local = 8t + j, and the
    analogous k~=(j', u), k = 8u + j'.

Softmax has no max-subtraction (scores are O(6) so exp cannot overflow) and
the mask is applied multiplicatively after exp (exact: masked ref scores are
-1e9 whose exp underflows to 0).

Scores are computed twice: once as S [q-part, k-free] for the attn output
(natural, contiguous stores) and once as S^T [k-part, q-free] to feed the
attn@V contraction (PE contracts over the partition dim). This is cheaper
than any transpose path for the 16.8M-element P matrix per core.
"""

import sys

sys.path.insert(0, "/opt/trn_rl_repo")

import numpy as np

import concourse.bass as bass
import concourse.mybir as mybir
import concourse.tile as tile
from concourse import bacc
from concourse.bass import ds
from concourse.masks import make_identity

FP32 = mybir.dt.float32
BF16 = mybir.dt.bfloat16
U8 = mybir.dt.uint8
AF = mybir.ActivationFunctionType
OP = mybir.AluOpType

H = 8          # heads
DH = 64        # head dim
DM = 512       # model dim
SQ = 1024      # query rows per core
SK = 2048      # key rows
LN_EPS = 1e-5
SCALE = 0.125  # 1/sqrt(64)


def build(nc: bass.Bass):
    xq_proj = nc.dram_tensor("xq_proj", [SQ, DM], FP32, kind="ExternalInput")
    xq_resid = nc.dram_tensor("xq_resid", [SQ, DM], FP32, kind="ExternalInput")
    xk = nc.dram_tensor("xk", [SK, DM], FP32, kind="ExternalInput")
    xv = nc.dram_tensor("xv", [SK, DM], FP32, kind="ExternalInput")
    maskh = nc.dram_tensor("maskh", [SQ, SK], U8, kind="ExternalInput")
    wq = nc.dram_tensor("wq", [DM, DM], FP32, kind="ExternalInput")
    wk = nc.dram_tensor("wk", [DM, DM], FP32, kind="ExternalInput")
    wfc = nc.dram_tensor("wfc", [DM, DM], FP32, kind="ExternalInput")
    gamma = nc.dram_tensor("gamma", [DM, 1], FP32, kind="ExternalInput")
    beta = nc.dram_tensor("beta", [DM, 1], FP32, kind="ExternalInput")
    attn_o = nc.dram_tensor("attn_o", [H, SQ, SK], FP32, kind="ExternalOutput")
    normed_o = nc.dram_tensor("normed_o", [SQ, DM], FP32, kind="ExternalOutput")

    with tile.TileContext(nc) as tc:
        _body(nc, tc, xq_proj, xq_resid, xk, xv, maskh, wq, wk, wfc,
              gamma, beta, attn_o, normed_o)
    return nc


def _body(nc, tc, xq_proj, xq_resid, xk, xv, maskh, wq, wk, wfc,
          gamma, beta, attn_o, normed_o):
    import contextlib
    ctx = contextlib.ExitStack()
    with ctx:
        cst = ctx.enter_context(tc.tile_pool(name="cst", bufs=1))
        work = ctx.enter_context(tc.tile_pool(name="work", bufs=2))
        head = ctx.enter_context(tc.tile_pool(name="head", bufs=2))
        ps_big = ctx.enter_context(tc.tile_pool(name="psb", bufs=2, space="PSUM"))
        ps_small = ctx.enter_context(tc.tile_pool(name="pss", bufs=4, space="PSUM"))

        # ---------------- constants ----------------
        id_bf = cst.tile([128, 128], BF16)
        id_f32 = cst.tile([128, 128], FP32)
        make_identity(nc, id_bf)
        make_identity(nc, id_f32)
        ones_col = cst.tile([128, 1], FP32)   # lhsT for partition-sum
        nc.vector.memset(ones_col, 1.0)
        ones_row = cst.tile([1, 128], FP32)   # lhsT for partition-broadcast
        nc.vector.memset(ones_row, 1.0)

        g_sb = cst.tile([128, 4, 1], FP32)
        b_sb = cst.tile([128, 4, 1], FP32)
        for cc in range(4):
            nc.sync.dma_start(g_sb[:, cc, :], gamma[ds(cc * 128, 128), :])
            nc.sync.dma_start(b_sb[:, cc, :], beta[ds(cc * 128, 128), :])

        # ---------------- weight transposes (bf16) ----------------
        def load_wT(w_dram, name):
            # wT[m, mc, o]
            wT = cst.tile([128, 4, DM], BF16, tag=name, name=name)
            for oc in range(4):
                w_natf = work.tile([128, DM], FP32, tag="natf", name="w_natf")
                nc.sync.dma_start(w_natf, w_dram[ds(oc * 128, 128), :])
                w_nat = work.tile([128, DM], BF16, tag="nat", name="w_nat")
                nc.vector.tensor_copy(w_nat, w_natf)
                psT = ps_small.tile([128, 4, 128], BF16, tag="small", name="psT")
                for mc in range(4):
                    nc.tensor.transpose(psT[:, mc, :], w_nat[:, ds(mc * 128, 128)],
                                        id_bf)
                nc.vector.tensor_copy(wT[:, :, ds(oc * 128, 128)], psT)
            return wT

        wqT = load_wT(wq, "wqT")
        wkT = load_wT(wk, "wkT")
        wfcT = load_wT(wfc, "wfcT")

        # ---------------- mask prep ----------------
        # mask01[t, j, k] = 1 - mask  (bf16); q~ tile j holds rows s=8t+j
        m01 = cst.tile([128, H, SK], BF16)
        maskh_v = maskh.rearrange("(t j) k -> t j k", j=8)
        for j in range(H):
            mu8 = work.tile([128, SK], U8, tag="pt_raw", name="mu8")
            nc.sync.dma_start(mu8, maskh_v[:, j, :])
            mbf = work.tile([128, SK], BF16, tag="p_raw", name="mbf")
            nc.vector.tensor_copy(mbf, mu8)
            nc.vector.tensor_scalar(m01[:, j, :], mbf, -1.0, 1.0, OP.mult, OP.add)

        # maskT01 [u, kt, q~]  (kt = j'*2 + ut)
        maskT = cst.tile([128, 16, SQ], BF16)
        for jp in range(8):
            for ut in range(2):
                kt = jp * 2 + ut
                psT = ps_small.tile([128, 8, 128], BF16, tag="small", name="psT")
                for j in range(8):
                    srcv = m01[:, j, :].rearrange("t (u jp) -> t jp u", jp=8)
                    nc.tensor.transpose(
                        psT[:, j, :], srcv[:, jp, ds(ut * 128, 128)], id_bf)
                nc.vector.tensor_copy(maskT[:, kt, :], psT)

        # ---------------- per-core persistent ----------------
        ctxT = cst.tile([128, 4, SQ], BF16)     # context^T [c, cc, q~]
        xT = cst.tile([128, 4, SQ], FP32)       # fc out + residual, [c, cc, q~]

        xq_proj_v = xq_proj.rearrange("(h t) c -> h t c", h=8)
        xk_v = xk.rearrange("(h r t) c -> h r t c", h=8, r=2)
        xv_v = xv.rearrange("(h r t) c -> h r t c", h=8, r=2)

        # ================= head loop =================
        for h in range(H):
            # ---- transposed input blocks ----
            xqT = head.tile([128, 4, 128], BF16, tag="xqT", name="xqT", bufs=1)
            xq_natf = work.tile([128, DM], FP32, tag="natf", name="xq_natf")
            nc.sync.dma_start(xq_natf, xq_proj_v[h])
            xq_nat = work.tile([128, DM], BF16, tag="nat", name="xq_nat")
            nc.vector.tensor_copy(xq_nat, xq_natf)
            psT = ps_small.tile([128, 4, 128], BF16, tag="small", name="psT")
            for mc in range(4):
                nc.tensor.transpose(psT[:, mc, :], xq_nat[:, ds(mc * 128, 128)],
                                    id_bf)
            nc.vector.tensor_copy(xqT, psT)

            xkT = head.tile([128, 4, 256], BF16, tag="xkT", name="xkT", bufs=1)
            xvT = head.tile([128, 4, 256], BF16, tag="xvT", name="xvT", bufs=1)
            for src_v, dstT in ((xk_v, xkT), (xv_v, xvT)):
                for rt in range(2):
                    natf = work.tile([128, DM], FP32, tag="natf", name="kv_natf")
                    nc.sync.dma_start(natf, src_v[h, rt])
                    nat = work.tile([128, DM], BF16, tag="nat", name="kv_nat")
                    nc.vector.tensor_copy(nat, natf)
                    psT = ps_small.tile([128, 4, 128], BF16, tag="small",
                                        name="psT")
                    for mc in range(4):
                        nc.tensor.transpose(psT[:, mc, :],
                                            nat[:, ds(mc * 128, 128)], id_bf)
                    nc.vector.tensor_copy(dstT[:, :, ds(rt * 128, 128)], psT)

            # ---- projections (64-partition tiles, base 0, j in free dim) ----
            QpT = head.tile([64, 8, 128], BF16, tag="QpT", name="QpT")
            for j in range(8):
                ps = ps_small.tile([64, 128], FP32, tag="small", name="ps_q")
                for mc in range(4):
                    nc.tensor.matmul(ps, wqT[:, mc, ds(j * 64, 64)],
                                     xqT[:, mc, :],
                                     start=(mc == 0), stop=(mc == 3))
                nc.vector.tensor_copy(QpT[:, j, :], ps)

            KpT = head.tile([64, 8, 256], BF16, tag="KpT", name="KpT")
            for jp in range(8):
                ps = ps_small.tile([64, 256], FP32, tag="small", name="ps_k")
                for mc in range(4):
                    nc.tensor.matmul(ps, wkT[:, mc, ds(jp * 64, 64)],
                                     xkT[:, mc, :],
                                     start=(mc == 0), stop=(mc == 3))
                nc.vector.tensor_copy(KpT[:, jp, :], ps)

            Vp = head.tile([128, 2, DM], BF16, tag="Vp", name="Vp")
            for ut in range(2):
                ps = ps_big.tile([128, 1024], FP32, tag="big", name="ps_v")
                for mc in range(4):
                    nc.tensor.matmul(ps[:, 0:DM], xvT[:, mc, ds(ut * 128, 128)],
                                     wkT[:, mc, :],
                                     start=(mc == 0), stop=(mc == 3))
                nc.vector.tensor_copy(Vp[:, ut, :], ps[:, 0:DM])

            vones = head.tile([128, 2, 8, 65], BF16, tag="vones", name="vones", bufs=1)
            nc.vector.memset(vones[:, :, :, 64:65], 1.0)
            for ut in range(2):
                nc.vector.tensor_copy(
                    vones[:, ut, :, 0:64],
                    Vp[:, ut, :].rearrange("u (jp d) -> u jp d", jp=8))

            # ---- S^T phase: PT_m[u, kt, q~] = exp(S^T)*maskT ----
            PT_m = head.tile([128, 16, SQ], BF16, tag="PT_m", name="PT_m",
                             bufs=1)
            for jp in range(8):
                for ut in range(2):
                    kt = jp * 2 + ut
                    ps = ps_big.tile([128, 1024], FP32, tag="big", name="ps_st")
                    for j in range(8):
                        nc.tensor.matmul(
                            ps[:, ds(j * 128, 128)],
                            KpT[:, jp, ds(ut * 128, 128)],
                            QpT[:, j, :],
                            start=True, stop=True)
                    pt_raw = work.tile([128, SQ], BF16, tag="pt_raw",
                                       name="pt_raw")
                    nc.scalar.activation(pt_raw, ps, AF.Exp, scale=SCALE)
                    nc.vector.tensor_tensor(PT_m[:, kt, :], pt_raw,
                                            maskT[:, kt, :], OP.mult)

            # ---- AV + denominators + context^T ----
            recs = head.tile([128, 8, 1], FP32, tag="recs", name="recs")
            for j in range(8):
                ps_av = ps_small.tile([128, 65], FP32, tag="small", name="ps_av")
                for kt in range(16):
                    jp, ut = kt // 2, kt % 2
                    nc.tensor.matmul(ps_av, PT_m[:, kt, ds(j * 128, 128)],
                                     vones[:, ut, jp, :],
                                     start=(kt == 0), stop=(kt == 15))
                dcol = work.tile([128, 1], FP32, tag="dcol", name="dcol")
                nc.vector.tensor_copy(dcol, ps_av[:, 64:65])
                nc.vector.reciprocal(recs[:, j, :], dcol)
                ctx_t = work.tile([128, 64], BF16, tag="ctx_t", name="ctx_t")
                nc.vector.tensor_scalar(ctx_t, ps_av[:, 0:64], recs[:, j, :],
                                        None, OP.mult)
                hp = 64 * (h % 2)
                psT = ps_small.tile([128, 128], BF16, tag="small", name="psTc")
                nc.tensor.transpose(psT[hp:hp + 64, :], ctx_t, id_bf,
                                    tile_position=(0, hp))
                nc.vector.tensor_copy(
                    ctxT[hp:hp + 64, h // 2, ds(j * 128, 128)],
                    psT[hp:hp + 64, :])

            # ---- S phase (natural-k): attn rows out ----
            for j in range(8):
                p_raw = work.tile([128, SK], BF16, tag="p_raw", name="p_raw")
                p_raw_v = p_raw.rearrange("t (u jp) -> t jp u", jp=8)
                for half in range(2):
                    ps = ps_big.tile([128, 1024], FP32, tag="big", name="ps_s")
                    for jj in range(4):
                        jp = half * 4 + jj
                        nc.tensor.matmul(
                            ps[:, ds(jj * 256, 256)],
                            QpT[:, j, :],
                            KpT[:, jp, :],
                            start=True, stop=True)
                    ps_v = ps.rearrange("t (jj u) -> t jj u", jj=4)
                    nc.scalar.activation(p_raw_v[:, ds(half * 4, 4), :], ps_v,
                                         AF.Exp, scale=SCALE)
                # fused: attn = (p_raw * recip) * mask01, bf16
                p_mn = work.tile([128, SK], BF16, tag="p_mn", name="p_mn")
                nc.vector.scalar_tensor_tensor(p_mn, p_raw, recs[:, j, :],
                                               m01[:, j, :], OP.mult, OP.mult)
                # cast-store bf16 -> f32 via SWDGE
                nc.gpsimd.dma_start(
                    attn_o[h].rearrange("(t j) k -> t j k", j=8)[:, j, :],
                    p_mn)

        # ================= epilogue =================
        # residual rows, transposed: xqrT [c, cc, q~] (tag-shares PT_m slot)
        xqrT = head.tile([128, 4, SQ], BF16, tag="PT_m", name="xqrT", bufs=1)
        xqr_v = xq_resid.rearrange("(t j) c -> t j c", j=8)
        for j in range(8):
            xr_natf = work.tile([128, DM], FP32, tag="natf", name="xr_natf")
            nc.sync.dma_start(xr_natf, xqr_v[:, j, :])
            xr_nat = work.tile([128, DM], BF16, tag="nat", name="xr_nat")
            nc.vector.tensor_copy(xr_nat, xr_natf)
            psT = ps_small.tile([128, 4, 128], BF16, tag="small", name="psT")
            for cc in range(4):
                nc.tensor.transpose(psT[:, cc, :], xr_nat[:, ds(cc * 128, 128)],
                                    id_bf)
            nc.vector.tensor_copy(xqrT[:, :, ds(j * 128, 128)], psT)

        # fc + residual -> xT
        for oc in range(4):
            ps = ps_big.tile([128, 1024], FP32, tag="big", name="ps_fc")
            for cc in range(4):
                for qh in range(2):
                    nc.tensor.matmul(ps[:, ds(qh * 512, 512)],
                                     wfcT[:, cc, ds(oc * 128, 128)],
                                     ctxT[:, cc, ds(qh * 512, 512)],
                                     start=(cc == 0), stop=(cc == 3))
            nc.vector.tensor_tensor(xT[:, oc, :], ps, xqrT[:, oc, :], OP.add)

        # LN stats: sum(x), sum(x^2) over c via ones-matmul (interleaved groups)
        mv = cst.tile([1, SQ], FP32)
        var = cst.tile([1, SQ], FP32)
        rstd = cst.tile([1, SQ], FP32)
        ps_s1a = ps_small.tile([1, 512], FP32, tag="small", name="ps_s1a")
        ps_s1b = ps_small.tile([1, 512], FP32, tag="small", name="ps_s1b")
        ps_s2a = ps_small.tile([1, 512], FP32, tag="small", name="ps_s2a")
        ps_s2b = ps_small.tile([1, 512], FP32, tag="small", name="ps_s2b")
        for cc in range(4):
            x2c = work.tile([128, SQ], FP32, tag="x2c", name="x2c")
            nc.vector.tensor_tensor(x2c, xT[:, cc, :], xT[:, cc, :], OP.mult)
            st = (cc == 0)
            sp = (cc == 3)
            nc.tensor.matmul(ps_s1a, ones_col, xT[:, cc, 0:512], start=st, stop=sp)
            nc.tensor.matmul(ps_s1b, ones_col, xT[:, cc, 512:1024], start=st,
                             stop=sp)
            nc.tensor.matmul(ps_s2a, ones_col, x2c[:, 0:512], start=st, stop=sp)
            nc.tensor.matmul(ps_s2b, ones_col, x2c[:, 512:1024], start=st,
                             stop=sp)
        nc.vector.tensor_scalar(mv[:, 0:512], ps_s1a, 1.0 / DM, None, OP.mult)
        nc.vector.tensor_scalar(mv[:, 512:1024], ps_s1b, 1.0 / DM, None, OP.mult)
        nc.vector.tensor_scalar(var[:, 0:512], ps_s2a, 1.0 / DM, None, OP.mult)
        nc.vector.tensor_scalar(var[:, 512:1024], ps_s2b, 1.0 / DM, None,
                                OP.mult)
        m2 = work.tile([1, SQ], FP32, tag="m2", name="m2", bufs=1)
        nc.vector.tensor_tensor(m2, mv, mv, OP.mult)
        nc.vector.tensor_tensor(var, var, m2, OP.subtract)
        nc.vector.tensor_scalar(var, var, LN_EPS, None, OP.add)
        sqv = work.tile([1, SQ], FP32, tag="m2", name="sqv", bufs=1)
        nc.scalar.activation(sqv, var, AF.Sqrt)
        nc.vector.reciprocal(rstd, sqv)

        # broadcast mean/rstd across partitions via PE outer product
        ps_bm = ps_big.tile([128, 1024], FP32, tag="big", name="ps_bm")
        ps_br = ps_big.tile([128, 1024], FP32, tag="big", name="ps_br")
        for qh in range(2):
            nc.tensor.matmul(ps_bm[:, ds(qh * 512, 512)], ones_row,
                             mv[:, ds(qh * 512, 512)], start=True, stop=True)
            nc.tensor.matmul(ps_br[:, ds(qh * 512, 512)], ones_row,
                             rstd[:, ds(qh * 512, 512)], start=True, stop=True)

        # apply LN, transpose back, store
        normed_v = normed_o.rearrange("(t j) c -> t j c", j=8)
        for cc in range(4):
            t1 = work.tile([128, SQ], FP32, tag="x2c", name="t1")
            nc.vector.tensor_tensor(t1, xT[:, cc, :], ps_bm, OP.subtract)
            nc.vector.tensor_tensor(t1, t1, ps_br, OP.mult)
            nc.vector.tensor_scalar(t1, t1, g_sb[:, cc, :], b_sb[:, cc, :],
                                    OP.mult, OP.add)
            for jh in range(2):
                psTn = ps_small.tile([128, 4, 128], FP32, tag="small",
                                     name="psTn")
                for jj in range(4):
                    j = jh * 4 + jj
                    nc.tensor.transpose(psTn[:, jj, :], t1[:, ds(j * 128, 128)],
                                        id_f32)
                nn = work.tile([128, 4, 128], FP32, tag="x2c", name="nn")
                nc.vector.tensor_copy(nn, psTn)
                nc.sync.dma_start(
                    normed_v[:, ds(jh * 4, 4), ds(cc * 128, 128)], nn)


# ---------------------------------------------------------------------------
# host side
# ---------------------------------------------------------------------------
_CACHED = {}


def _get_nc():
    if "nc" not in _CACHED:
        nc = bacc.Bacc("TRN2", target_bir_lowering=False)
        build(nc)
        nc.compile()
        _CACHED["nc"] = nc
    return _CACHED["nc"]


def kernel(input_Q, input_K, input_V, attention_mask, W_Q, W_K, W_V, W_fc,
           ln_gamma, ln_beta):
    from concourse import bass_utils

    input_Q = np.asarray(input_Q, dtype=np.float32)
    input_K = np.asarray(input_K, dtype=np.float32)
    input_V = np.asarray(input_V, dtype=np.float32)
    mask_u8 = np.asarray(attention_mask).astype(np.uint8)
    W_Q = np.ascontiguousarray(np.asarray(W_Q, dtype=np.float32))
    W_K = np.ascontiguousarray(np.asarray(W_K, dtype=np.float32))
    W_fc = np.ascontiguousarray(np.asarray(W_fc, dtype=np.float32))
    gamma = np.ascontiguousarray(
        np.asarray(ln_gamma, dtype=np.float32).reshape(DM, 1))
    beta = np.ascontiguousarray(
        np.asarray(ln_beta, dtype=np.float32).reshape(DM, 1))

    nc = _get_nc()
    in_maps = []
    for c in range(8):
        b, sh = c // 2, c % 2
        xq = input_Q[b]
        in_maps.append({
            "xq_proj": np.ascontiguousarray(
                xq.reshape(8, 2, 128, DM)[:, sh].reshape(SQ, DM)),
            "xq_resid": np.ascontiguousarray(xq[sh * SQ:(sh + 1) * SQ]),
            "xk": np.ascontiguousarray(input_K[b]),
            "xv": np.ascontiguousarray(input_V[b]),
            "maskh": np.ascontiguousarray(mask_u8[b, sh * SQ:(sh + 1) * SQ]),
            "wq": W_Q, "wk": W_K, "wfc": W_fc,
            "gamma": gamma, "beta": beta,
        })

    trace = bool(globals().get("_TRACE", False))
    kw = {}
    if trace:
        kw = dict(trace=True, tmpdir="/root/problem/profile")
    res = bass_utils.run_bass_kernel_spmd(nc, in_maps, core_ids=list(range(8)),
                                          **kw)
    results = res.results
    _CACHED["exec_time_ns"] = res.exec_time_ns
    _CACHED["profile_json"] = getattr(res, "profile_json", None)

    B, S = 4, 2048
    normed = np.empty((B, S, DM), dtype=np.float32)
    attn = np.empty((B, H, S, S), dtype=np.float32)
    for c in range(8):
        b, sh = c // 2, c % 2
        normed[b, sh * SQ:(sh + 1) * SQ] = results[c]["normed_o"]
        attn[b, :, sh * SQ:(sh + 1) * SQ, :] = results[c]["attn_o"]
    return normed, attn
